# revision 8
# baseline (speedup 1.0000x reference)
"""Trainium2 Bass kernel for BDH recurrent (chunked linear) attention.

Problem shapes (hardcoded): Q_raw [2,16,2048,256] f32, V_raw [2,2048,1024] f32,
out [2,16,2048,1024] f32.  8 NeuronCores, data+head parallel: each core owns
4 (batch, head) pairs; V is shared across the 4 heads of a core's batch.

Math (reference semantics), per (b,h), chunks of 128:
  QR = rope(Q); KR = QR
  out_c = q_c @ state_{<c} + (q_c q_c^T  * strict_tril) v_c
  state += q_c^T v_c

This version uses superchunks of SUP=2 chunks: per-pair PE cycles are
6144*SUP + 75776 -> 88064 at SUP=2 (the PE-work optimum subject to the
PSUM-evacuation bandwidth of DVE+ScalarE; SUP=1 would be 81920 cycles but
its per-chunk state casts exceed the 1x-rate PSUM-source copy bandwidth).

Per superchunk s (chunks a=2s, b=2s+1), PE order:
  PV_a | m1_a | G(next sup, hoisted) | m4_a | PV_b | m1_b | m4_b
out PSUM banks are single-buffered (out_a evacuated during G'/m4_a, out_b
split DVE/ACT during m4_b), the fp32 PSUM state is cast to SBUF bf16 in
four [128,512] quarter-copies right after each m4 bank group closes
(DVE: m0h0; ACT: m0h1, m1h0, m1h1 -- each lands a few hundred ns before
the next superchunk's m1 consumes it), and G is hoisted one superchunk so
its SBUF evacuation is never on the PE critical path.

RoPE runs on DVE in both layouts (natural [t,n] for the m4 lhsT, transposed
[n,t] for the G/m1 lhsT), pair-deinterleaved host-side (planes=evens|odds,
6 tensor_tensor ops per layout).  Each pair's chunks 0-1 natural layout
comes from PE-transposing qrT (the rope tail would arrive too late);
chunks 14-15 natural are never needed (their state update is dead).  The
transposed rope for pair bh+1 drains through pair bh's DVE slack 1 op per
superchunk; the natural rope self-paces 1 two-chunk block per superchunk,
one superchunk ahead of its m4.  All DRAM layouts are partition-major;
output is written partition-major bf16 and un-permuted on host.
"""

import numpy as np
import ml_dtypes

import concourse.mybir as mybir
import concourse.tile as tile
from concourse import bacc
from concourse.bass import ds
from concourse.bass_utils import run_bass_kernel_spmd
from concourse.masks import make_identity

B, NH, T, N, D = 2, 16, 2048, 256, 1024
P = 128          # partition / chunk size
NCH = T // P     # 16 chunks
SUP = 2          # chunks per superchunk
NSUP = NCH // SUP  # 8
HPC = 4          # (b,h) pairs per core
NCORES = 8
THETA = 2.0 ** 16
TWO_PI = 2.0 * np.pi

bf = mybir.dt.bfloat16
f32 = mybir.dt.float32
bf_np = ml_dtypes.bfloat16

mult = mybir.AluOpType.mult
add_op = mybir.AluOpType.add
sub_op = mybir.AluOpType.subtract


def _emit_body(nc, tc, qn, qt, v, cn, sn, ct, st, mskT, out):
    """Tile program for one core: 4 (b,h) pairs, full scan each."""
    with (
        tc.tile_pool(name="const", bufs=1) as constp,
        tc.tile_pool(name="qpool", bufs=2) as qpool,
        tc.tile_pool(name="gpool", bufs=2) as gpool,
        tc.tile_pool(name="tmppool", bufs=2) as tmpp,
        tc.tile_pool(name="outbuf", bufs=1) as outp,
        tc.tile_pool(name="statesb", bufs=1) as statep,
        tc.tile_pool(name="ps_state", bufs=1, space="PSUM") as ps_state,
        tc.tile_pool(name="ps_out", bufs=1, space="PSUM") as ps_out,
        tc.tile_pool(name="ps_g", bufs=1, space="PSUM") as ps_g,
        tc.tile_pool(name="ps_t", bufs=1, space="PSUM") as ps_t,
    ):
        # resident constants (all DRAM layouts partition-major/contiguous).
        # DMAs are spread across engine queues so the transfers that gate the
        # startup ramp (ct+qt0 -> first rope op -> G_0) run in parallel.
        msk_sb = constp.tile([P, 3 * P], bf)
        ct_sb = constp.tile([P, T], bf)
        nc.sync.dma_start(ct_sb[:], ct[:, :])
        st_sb = constp.tile([P, T], bf)
        nc.gpsimd.dma_start(st_sb[:], st[:, :])
        nc.gpsimd.dma_start(msk_sb[:], mskT[:, :])
        ident = constp.tile([P, P], bf)
        make_identity(nc, ident)
        cn_sb = constp.tile([P, NCH, P], bf)
        sn_sb = constp.tile([P, NCH, P], bf)
        v_sb = constp.tile([P, NCH, D], bf)

        def alloc_pair():
            return {
                "qt": qpool.tile([P, 2, T], bf, tag="qt", name="qt_sb"),
                "qn": qpool.tile([P, 2, NCH, P], bf, tag="qn", name="qn_sb"),
                "qrT": qpool.tile([P, 2, T], bf, tag="qrT", name="qrT"),
                "qr": qpool.tile([P, 2, NCH, P], bf, tag="qr", name="qr"),
                "tmpT": tmpp.tile([P, 2, T], bf, tag="tmpT", name="tmpT"),
                "tmpN": tmpp.tile([P, 2, NCH, P], bf, tag="tmpN", name="tmpN"),
            }

        def dma_pair(bh, pr):
            eng = nc.scalar if bh == 0 else nc.gpsimd
            eng.dma_start(pr["qt"][:, 0], qt[bh, 0])
            eng.dma_start(pr["qt"][:, 1], qt[bh, 1])
            eng.dma_start(pr["qn"][:], qn[bh])

        # ---- rope op factories: each returns a list of DVE-op closures ----
        def ropeT_ops(pr, t0, w):
            sl = ds(t0, w)
            q, tm, qi = pr["qrT"], pr["tmpT"], pr["qt"]
            return [
                lambda: nc.vector.tensor_tensor(
                    q[:, 0, sl], qi[:, 0, sl], ct_sb[:, sl], mult),
                lambda: nc.vector.tensor_tensor(
                    tm[:, 0, sl], qi[:, 1, sl], st_sb[:, sl], mult),
                lambda: nc.vector.tensor_tensor(
                    q[:, 0, sl], q[:, 0, sl], tm[:, 0, sl], sub_op),
                lambda: nc.vector.tensor_tensor(
                    q[:, 1, sl], qi[:, 1, sl], ct_sb[:, sl], mult),
                lambda: nc.vector.tensor_tensor(
                    tm[:, 1, sl], qi[:, 0, sl], st_sb[:, sl], mult),
                lambda: nc.vector.tensor_tensor(
                    q[:, 1, sl], q[:, 1, sl], tm[:, 1, sl], add_op),
            ]

        def ropeN_ops(pr, c0, nch):
            csl = ds(c0, nch)
            q, tm, qi = pr["qr"], pr["tmpN"], pr["qn"]
            return [
                lambda: nc.vector.tensor_tensor(
                    q[:, 0, csl], qi[:, 0, csl], cn_sb[:, csl], mult),
                lambda: nc.vector.tensor_tensor(
                    tm[:, 0, csl], qi[:, 1, csl], sn_sb[:, csl], mult),
                lambda: nc.vector.tensor_tensor(
                    q[:, 0, csl], q[:, 0, csl], tm[:, 0, csl], sub_op),
                lambda: nc.vector.tensor_tensor(
                    q[:, 1, csl], qi[:, 1, csl], cn_sb[:, csl], mult),
                lambda: nc.vector.tensor_tensor(
                    tm[:, 1, csl], qi[:, 0, csl], sn_sb[:, csl], mult),
                lambda: nc.vector.tensor_tensor(
                    q[:, 1, csl], q[:, 1, csl], tm[:, 1, csl], add_op),
            ]

        def drain(queue, nops):
            for _ in range(min(nops, len(queue))):
                queue.pop(0)()

        # qr (natural layout) chunks 0-1 via PE transpose of qrT
        def emit_transpose(pr, c2, m):
            t_ps = ps_t.tile([P, P], bf, tag="tps", name="t_ps")
            nc.tensor.transpose(t_ps[:], pr["qrT"][:, m, ds(c2 * P, P)],
                                ident[:])
            nc.vector.tensor_copy(pr["qr"][:, m, c2, :], t_ps[:])

        # G for superchunk (chunks a, a+1) -> PSUM + SBUF tiles.
        # g0 [P, 2P]: cols 0..P masked diag of chunk a (strict-triu in the
        # transposed layout), cols P..2P cross block (chunk-a rows vs
        # chunk-b cols).  g1 [P, P]: masked diag of chunk b.
        def emit_G(pr, a):
            qrT = pr["qrT"]
            g_ps = ps_g.tile([P, 512], f32, tag="g", name="g_ps")
            nc.tensor.matmul(
                g_ps[:, 0:256], qrT[:, 0, ds(a * P, P)],
                qrT[:, 0, ds(a * P, 2 * P)], start=True, stop=False)
            nc.tensor.matmul(
                g_ps[:, 0:256], qrT[:, 1, ds(a * P, P)],
                qrT[:, 1, ds(a * P, 2 * P)], start=False, stop=True)
            nc.tensor.matmul(
                g_ps[:, 256:384], qrT[:, 0, ds((a + 1) * P, P)],
                qrT[:, 0, ds((a + 1) * P, P)], start=True, stop=False,
                skip_group_check=True)
            nc.tensor.matmul(
                g_ps[:, 256:384], qrT[:, 1, ds((a + 1) * P, P)],
                qrT[:, 1, ds((a + 1) * P, P)], start=False, stop=True,
                skip_group_check=True)
            g_sb = gpool.tile([P, 3 * P], bf, tag="g", name="g_sb")
            return g_ps, g_sb

        def evac_G(g_ps, g_sb):
            # single op; mask is [strict-triu | ones | strict-triu]
            nc.vector.tensor_tensor(g_sb[:], g_ps[:, 0:384], msk_sb[:, :384], mult)

        out_ring = outp.tile([P, 4, D], bf, name="out_ring")
        state_ps = ps_state.tile([P, 2, D], f32, name="state_ps")
        state_sb = statep.tile([P, 2, D], bf, name="state_sb")
        out_ps = ps_out.tile([P, 2, 512], f32, name="out_ps")

        # ---- startup: pair 0 ----
        pairs = [None, None]
        pr0 = alloc_pair()
        pairs[0] = pr0
        dma_pair(0, pr0)
        # remaining input DMAs, ordered so early superchunks aren't gated
        nc.gpsimd.dma_start(cn_sb[:], cn[:, :, :])
        nc.gpsimd.dma_start(sn_sb[:], sn[:, :, :])
        nc.sync.dma_start(v_sb[:, :SUP], v[:, :SUP, :])
        nc.sync.dma_start(v_sb[:, SUP:6], v[:, SUP:6, :])
        nc.sync.dma_start(v_sb[:, 6:10], v[:, 6:10, :])
        nc.sync.dma_start(v_sb[:, 10:], v[:, 10:, :])

        # transposed rope, chunks 0-3 (gates G_0 and the hoisted G of sup 1)
        for op in ropeT_ops(pr0, 0, 2 * P):
            op()
        for op in ropeT_ops(pr0, 2 * P, 2 * P):
            op()
        for c2 in range(2):
            for m in range(2):
                emit_transpose(pr0, c2, m)

        # backlog ownT: pair 0's remaining transposed rope (2-chunk blocks,
        # drained 1 block/sup one sup ahead of the hoisted G that needs it).
        ownT = []
        for k in range(2, NSUP):
            ownT.extend(ropeT_ops(pr0, 2 * k * P, 2 * P))

        g_cur = emit_G(pr0, 0)
        evac_G(*g_cur)

        for bh in range(HPC):
            pr = pairs[bh % 2]
            # ownN: this pair's natural rope (chunks 2..13 -- 0-1 come from
            # PE transposes, 14-15 are dead), 2-chunk blocks drained
            # 1 block/sup one sup ahead of their m4.
            ownN = []
            for k in range(1, 7):
                ownN.extend(ropeN_ops(pr, 2 * k, 2))
            nxt = None
            nxtT = []
            if bh + 1 < HPC:
                nxt = alloc_pair()
                pairs[(bh + 1) % 2] = nxt
                dma_pair(bh + 1, nxt)
                nxtT = ropeT_ops(nxt, 0, 2 * P) + ropeT_ops(nxt, 2 * P, T - 2 * P)

            for s in range(NSUP):
                a, b = SUP * s, SUP * s + 1
                _, g_sb = g_cur

                # ---- DVE backlog drains (queue position = sup top) ----
                drain(ownN, 6)          # one 2-chunk natural block
                drain(ownT, 6)          # pair 0 only: one transposed block
                if nxt is not None:
                    # next pair's transposed rope: chunks 0-1 as a small
                    # block at sup 1 (gates that pair's G_0 and its PE
                    # transposes), the rest as full-width ops.  Pair 0's
                    # own backlog still occupies sups 2-5, so its nxtT
                    # tail drains double at sups 6-7 instead.
                    if s == 1:
                        drain(nxtT, 6)
                    elif bh == 0:
                        if s in (4, 5):
                            drain(nxtT, 1)
                        elif s >= 6:
                            drain(nxtT, 2)
                    elif s >= 2:
                        drain(nxtT, 1)

                # ---- chunk a: PV_a, then the hoisted G (it buys the
                # state casts an extra 320ns before m1_a consumes them) ----
                for h in range(2):
                    nc.tensor.matmul(
                        out_ps[:, h, :], g_sb[:, 0:P], v_sb[:, a, ds(h * 512, 512)],
                        start=True, stop=(s == 0),
                        skip_group_check=True)

                g_nxt = None
                if s < NSUP - 1:
                    g_nxt = emit_G(pr, SUP * (s + 1))
                elif nxt is not None:
                    g_nxt = emit_G(nxt, 0)

                if s > 0:
                    for m in range(2):
                        for h in range(2):
                            nc.tensor.matmul(
                                out_ps[:, h, :], pr["qrT"][:, m, ds(a * P, P)],
                                state_sb[:, m, ds(h * 512, 512)],
                                start=False, stop=(m == 1),
                                skip_group_check=True)

                # next pair's chunk 0-1 natural layout via PE transpose
                if nxt is not None and 3 <= s <= 6:
                    k = s - 3
                    emit_transpose(nxt, k // 2, k % 2)

                # m4_a (state accumulate; skipped for the last superchunk)
                if s < NSUP - 1:
                    for m, h in ((0, 0), (0, 1), (1, 0), (1, 1)):
                        nc.tensor.matmul(
                            state_ps[:, m, ds(h * 512, 512)],
                            pr["qr"][:, m, a, :], v_sb[:, a, ds(h * 512, 512)],
                            start=(s == 0), stop=False,
                            skip_group_check=True)

                if g_nxt is not None:
                    evac_G(*g_nxt)
                # out_a evacuation (ScalarE, one [P, 1024] op)
                nc.scalar.copy(out_ring[:, a % 4, :], out_ps[:])

                # ---- chunk b: PV_b (cross + diag) + m1_b ----
                for h in range(2):
                    nc.tensor.matmul(
                        out_ps[:, h, :], g_sb[:, P:2 * P],
                        v_sb[:, a, ds(h * 512, 512)],
                        start=True, stop=False, skip_group_check=True)
                for h in range(2):
                    nc.tensor.matmul(
                        out_ps[:, h, :], g_sb[:, 2 * P:3 * P],
                        v_sb[:, b, ds(h * 512, 512)],
                        start=False, stop=(s == 0), skip_group_check=True)
                if s > 0:
                    # h-major: the h0 out bank closes 2 matmuls early, so
                    # its DVE evacuation overlaps the m4_b stream
                    for h in range(2):
                        for m in range(2):
                            nc.tensor.matmul(
                                out_ps[:, h, :], pr["qrT"][:, m, ds(b * P, P)],
                                state_sb[:, m, ds(h * 512, 512)],
                                start=False, stop=(m == 1),
                                skip_group_check=True)

                # out_b h0 evacuation can start as soon as its bank closed
                nc.vector.tensor_copy(out_ring[:, b % 4, ds(0, 512)],
                                      out_ps[:, 0, :])
                nc.scalar.copy(out_ring[:, b % 4, ds(512, 512)], out_ps[:, 1, :])

                # m4_b closes the per-sup state bank groups in cast order
                if s < NSUP - 1:
                    for m, h in ((0, 0), (0, 1), (1, 0), (1, 1)):
                        nc.tensor.matmul(
                            state_ps[:, m, ds(h * 512, 512)],
                            pr["qr"][:, m, b, :], v_sb[:, b, ds(h * 512, 512)],
                            start=False, stop=True,
                            skip_group_check=True)

                # state casts for the next superchunk's m1, quarter-granular
                # in m4's bank-close order (DVE: m0h0, m1h0; ACT: m0h1, m1h1)
                if s < NSUP - 1:
                    nc.vector.tensor_copy(state_sb[:, 0, ds(0, 512)],
                                          state_ps[:, 0, ds(0, 512)])
                    nc.vector.tensor_copy(state_sb[:, 1, ds(0, 512)],
                                          state_ps[:, 1, ds(0, 512)])
                    nc.scalar.copy(state_sb[:, 0, ds(512, 512)],
                                   state_ps[:, 0, ds(512, 512)])
                    nc.scalar.copy(state_sb[:, 1, ds(512, 512)],
                                   state_ps[:, 1, ds(512, 512)])

                nc.sync.dma_start(out[bh, :, ds(a, 2), :],
                                  out_ring[:, ds(a % 4, 2), :])

                g_cur = g_nxt


_BUILT = {}


def _build():
    if "nc" in _BUILT:
        return _BUILT["nc"]
    nc = bacc.Bacc(
        "TRN2", target_bir_lowering=False, debug=False,
        enable_asserts=True, num_devices=NCORES,
    )
    qn = nc.dram_tensor("qn", [HPC, P, 2, NCH, P], bf, kind="ExternalInput")
    qt = nc.dram_tensor("qt", [HPC, 2, P, T], bf, kind="ExternalInput")
    v = nc.dram_tensor("v", [P, NCH, D], bf, kind="ExternalInput")
    cn = nc.dram_tensor("cn", [P, NCH, P], bf, kind="ExternalInput")
    sn = nc.dram_tensor("sn", [P, NCH, P], bf, kind="ExternalInput")
    ct = nc.dram_tensor("ct", [P, T], bf, kind="ExternalInput")
    st = nc.dram_tensor("st", [P, T], bf, kind="ExternalInput")
    mskT = nc.dram_tensor("mskT", [P, 3 * P], bf, kind="ExternalInput")
    out = nc.dram_tensor("out", [HPC, P, NCH, D], bf, kind="ExternalOutput")
    with tile.TileContext(nc) as tc:
        _emit_body(nc, tc, qn, qt, v, cn, sn, ct, st, mskT, out)
    nc.compile()
    _BUILT["nc"] = nc
    return nc


def _host_prep(Q_raw, V_raw):
    """Shard + precompute device inputs (bf16, partition-major layouts)."""
    Q = np.asarray(Q_raw, dtype=np.float32)
    V = np.asarray(V_raw, dtype=np.float32)

    # rope tables, matching reference._get_freqs / _rope in float32
    t = np.arange(N, dtype=np.float32)
    q = np.floor(t / 2.0) * 2.0
    freqs = (1.0 / (THETA ** (q / np.float32(N))) / np.float32(TWO_PI)).astype(
        np.float32
    )
    phases = np.arange(T, dtype=np.float32)[:, None] * freqs[None, :]
    ph = (phases % 1.0) * np.float32(TWO_PI)
    # freqs are equal within each (even, odd) pair -> keep only even columns
    cosf = np.cos(ph[:, 0::2]).astype(bf_np)        # [T, 128]
    sinf = np.sin(ph[:, 0::2]).astype(bf_np)
    # natural tables [P, NCH, P]: (p, c, k) = table[c*128+p, k]
    cn = np.ascontiguousarray(cosf.reshape(NCH, P, P).transpose(1, 0, 2))
    sn = np.ascontiguousarray(sinf.reshape(NCH, P, P).transpose(1, 0, 2))
    # transposed tables [P, T]: (k, t)
    ct = np.ascontiguousarray(cosf.T)
    st = np.ascontiguousarray(sinf.T)
    mskT = np.ones((P, 3 * P), np.float32)
    mskT[:, :P] = np.triu(np.ones((P, P), np.float32), k=1)
    mskT[:, 2 * P:] = mskT[:, :P]
    mskT = mskT.astype(bf_np)

    # deinterleave pairs: planes (evens, odds), cast bf16
    Qd = np.stack([Q[..., 0::2], Q[..., 1::2]], axis=2).astype(bf_np)
    # Qd: [B, NH, 2, T, 128]
    # natural layout  [b,h][p, half, c, k] = Qd[b, h, half, c*128+p, k]
    Qn = np.ascontiguousarray(
        Qd.reshape(B, NH, 2, NCH, P, P).transpose(0, 1, 4, 2, 3, 5)
    )  # [B, NH, P, 2, NCH, P]
    # transposed layout [b,h][half, k, t] = Qd[b, h, half, t, k]
    Qt = np.ascontiguousarray(Qd.transpose(0, 1, 2, 4, 3))  # [B, NH, 2, 128, T]

    V16 = V.astype(bf_np)
    # v layout [P, NCH, D]: (p, c, d) = V[c*128+p, d]
    Vp = np.ascontiguousarray(V16.reshape(B, NCH, P, D).transpose(0, 2, 1, 3))

    in_maps = []
    for core in range(NCORES):
        b = core // (NCORES // B)
        hs = (core % (NCORES // B)) * HPC
        in_maps.append(
            {
                "qn": np.ascontiguousarray(Qn[b, hs : hs + HPC]),
                "qt": np.ascontiguousarray(Qt[b, hs : hs + HPC]),
                "v": Vp[b],
                "cn": cn,
                "sn": sn,
                "ct": ct,
                "st": st,
                "mskT": mskT,
            }
        )
    return in_maps


def _run(inputs, trace=False, **kw):
    nc = _build()
    in_maps = _host_prep(inputs["Q_raw"], inputs["V_raw"])
    res = run_bass_kernel_spmd(nc, in_maps, list(range(NCORES)), trace=trace, **kw)
    out = np.empty((B, NH, T, D), dtype=np.float32)
    for core in range(NCORES):
        b = core // (NCORES // B)
        hs = (core % (NCORES // B)) * HPC
        # device out: [HPC, P, NCH, D] partition-major -> [HPC, T, D]
        o = res.results[core]["out"].astype(np.float32)
        out[b, hs : hs + HPC] = o.transpose(0, 2, 1, 3).reshape(HPC, T, D)
    return out, res


def kernel(**inputs):
    out, _ = _run(inputs)
    return out
